# revision 7
# baseline (speedup 1.0000x reference)
"""Self-contained Trainium2 kernel for ReRoPE sparse attention.

Problem: x(2,1024,2048) -> attention with 16 Q heads / 8 KV heads (GQA),
RoPE within a 256-token causal band, ReRoPE (query rotated at fixed
position 256, keys unrotated) outside the band, -> out proj (2048x2048).

Sharding: 8 cores = 2 batches x 4 head groups. Each core computes 4 Q
heads / 2 KV heads of one batch plus its slice of all projections, and
produces a partial (1024,2048) output (wo row-parallel). Partials are
summed on the host (the per-batch all-reduce equivalent).

Score identity used: s2 = (R_W q)@k  ==  q @ (R_{-W} k), so the fixed
ReRoPE rotation is applied once to K instead of Q (q2 is just raw q).
Head dims are de-interleaved (evens|odds) via a host-side permutation of
wq/wk columns so RoPE pairs live on partitions (p, p+64).

v1: inputs packed host-side into one DRAM blob in consumption order
(masks/tables, then 8 chunk-pair groups of wk|wv|wq|x, then wo) and
streamed via ~12 large descriptors on two DMA rings, so the PE is dense
from the first data arrival instead of idling HAM-cold for ~27us on
late weights. K+V(half) projections are chunk-paced in wave A (8 psum
banks), V rest + Q run dense in wave B. Scores are computed
key-stationary with wide moving-query matmuls (one band MM N<=384 and
<=2 far MMs N<=512 per key block instead of per-128 tiles). RoPE is
fused to full-width [128,S] vector ops. Mask selects run on the
otherwise-idle GpSimd engine, writing exp tiles in place. A^T for the
out-projection is produced by SBUF->SBUF DMA transpose on the idle DMA
rings instead of PE transposes.

All device compute in bf16 (fp32 PSUM accumulation).
"""

import numpy as np
import ml_dtypes

B, S, D = 2, 1024, 2048
NH, NKV, HD = 16, 8, 128
W = 256
HPC, KPC = 4, 2            # q heads / kv heads per core
KC = D // 128              # 16 contraction chunks
SB = S // 128              # 8 sequence blocks
SCALE = 1.0 / float(np.sqrt(HD))
BF16 = ml_dtypes.bfloat16

# blob column offsets (bf16 [128, NBLOB])
O_M0 = 0
O_M2 = 128
O_TAB = 256                 # cos | sin_signed, 2*S cols
O_GRP = O_TAB + 2 * S       # 8 groups of (wk 512 | wv 512 | wq 1024 | x 2048)
GRPW = 4096
O_WO = O_GRP + 8 * GRPW     # 4 heads * 2048
NBLOB = O_WO + HPC * D

_NC_CACHE = {}


def _build_nc():
    import concourse.bass as bass
    import concourse.tile as tile
    from concourse import bacc, mybir
    from contextlib import ExitStack

    bf = mybir.dt.bfloat16
    f32 = mybir.dt.float32
    AF = mybir.ActivationFunctionType
    M = mybir.AluOpType

    nc = bacc.Bacc()
    blob = nc.declare_dram_parameter("blob", [128, NBLOB], bf, isOutput=False)
    cwd = nc.declare_dram_parameter("cw", [128, 3], f32, isOutput=False)
    out = nc.declare_dram_parameter("out", [S, D], bf, isOutput=True)

    with tile.TileContext(nc) as tc:
        with ExitStack() as ctx:
            p_in = ctx.enter_context(tc.tile_pool(name="p_in", bufs=1))
            p_q = ctx.enter_context(tc.tile_pool(name="p_q", bufs=2 * HPC))
            p_k = ctx.enter_context(tc.tile_pool(name="p_k", bufs=2 * KPC))
            p_v = ctx.enter_context(tc.tile_pool(name="p_v", bufs=SB))
            p_ao = ctx.enter_context(tc.tile_pool(name="p_ao", bufs=HPC))
            p_e = ctx.enter_context(tc.tile_pool(name="p_e", bufs=16))
            p_pt = ctx.enter_context(tc.tile_pool(name="p_pt", bufs=12))
            p_rt = ctx.enter_context(tc.tile_pool(name="p_rt", bufs=4))
            p_rc = ctx.enter_context(tc.tile_pool(name="p_rc", bufs=4))
            p_st = ctx.enter_context(tc.tile_pool(name="p_st", bufs=4))

            ps = ctx.enter_context(
                tc.tile_pool(name="ps", bufs=8, space="PSUM"))

            def pst(w, name):
                return ps.tile([128, w], f32, tag="ps", name=name,
                               padded_shape=[128, 512])

            # ---- input DMA: blob streamed in consumption order ----
            bsb = p_in.tile([128, NBLOB], bf, tag="blob")
            cw_sb = p_in.tile([128, 3], f32, tag="cw")
            nc.scalar.dma_start(cw_sb[:], cwd[:, :])
            nc.sync.dma_start(bsb[:, 0:O_GRP], blob[:, 0:O_GRP])
            for g in range(8):
                lo, hi = O_GRP + g * GRPW, O_GRP + (g + 1) * GRPW
                eng = nc.sync if g % 2 == 0 else nc.scalar
                eng.dma_start(bsb[:, lo:hi], blob[:, lo:hi])
            h_wo = O_WO + HPC * D // 2
            nc.sync.dma_start(bsb[:, O_WO:h_wo], blob[:, O_WO:h_wo])
            nc.scalar.dma_start(bsb[:, h_wo:NBLOB], blob[:, h_wo:NBLOB])

            m0_t = bsb[:, O_M0:O_M0 + 128]       # (k <= q)
            m2_t = bsb[:, O_M2:O_M2 + 128]       # (q < k)
            cosT = bsb[:, O_TAB:O_TAB + S]
            sinT = bsb[:, O_TAB + S:O_TAB + 2 * S]  # top half negated

            def wk_c(t):      # [128, 256] chunk t of wk
                g, e = divmod(t, 2)
                o = O_GRP + g * GRPW + e * 256
                return bsb[:, o:o + 256]

            def wv_c(t):
                g, e = divmod(t, 2)
                o = O_GRP + g * GRPW + 512 + e * 256
                return bsb[:, o:o + 256]

            def wq_c(t):      # [128, 512] chunk t of wq
                g, e = divmod(t, 2)
                o = O_GRP + g * GRPW + 1024 + e * 512
                return bsb[:, o:o + 512]

            def xts(t, lo, hi):
                g, e = divmod(t, 2)
                o = O_GRP + g * GRPW + 2048 + e * 1024
                return bsb[:, o + lo:o + hi]

            def wo_s(h, lo, hi):
                o = O_WO + h * D
                return bsb[:, o + lo:o + hi]

            def rope_var(dst, src):
                """Positional rope on full [128, S]; pairs on (p, p+64).
                sinT carries signs: rows 0:64 = -sin, 64:128 = +sin.
                TensorTensor needs co-based SB inputs, so the half-swap
                happens in the sin-product's OUTPUT placement."""
                tcs = p_rt.tile([128, S], bf, tag="rt")
                nc.vector.tensor_mul(tcs[:], src[:, 0:S], cosT)
                tsw = p_rt.tile([128, S], bf, tag="rt")
                nc.vector.tensor_mul(tsw[0:64, :], src[64:128, 0:S],
                                     sinT[64:128, :])   # +qi*sin
                nc.vector.tensor_mul(tsw[64:128, :], src[0:64, 0:S],
                                     sinT[0:64, :])     # -qr*sin
                # top: qr*cos - qi*sin ; bottom: qi*cos + qr*sin
                nc.vector.tensor_sub(dst[:, 0:S], tcs[:], tsw[:])

            def rope_negw(dst, src):
                """R_{-W}: or = r*cw + i*sw, oi = i*cw - r*sw.
                cw cols: 0 = cw, 1 = +sw, 2 = -sw (all partitions)."""
                tsw = p_rt.tile([128, S], bf, tag="rt")
                nc.vector.tensor_scalar_mul(tsw[0:64, :], src[64:128, 0:S],
                                            cw_sb[64:128, 1:2])  # ki*sw
                nc.vector.tensor_scalar_mul(tsw[64:128, :], src[0:64, 0:S],
                                            cw_sb[0:64, 2:3])    # -kr*sw
                nc.vector.scalar_tensor_tensor(
                    dst[:, 0:S], src[:, 0:S], cw_sb[:, 0:1],
                    tsw[:], M.mult, M.add)

            # ---- wave A: K chains + V chains sb0..3, chunk-paced ----
            k1_t = [p_k.tile([128, S], bf, tag="k", name=f"k1_{i}")
                    for i in range(KPC)]
            k2_t = [p_k.tile([128, S], bf, tag="k", name=f"k2_{i}")
                    for i in range(KPC)]
            kps = [pst(512, f"kps{i}") for i in range(4)]
            vps_a = [pst(256, f"vpsa{i}") for i in range(4)]
            for t in range(KC):
                for kv in range(KPC):
                    for half in range(2):
                        nc.tensor.matmul(
                            kps[kv * 2 + half][:],
                            lhsT=wk_c(t)[:, kv * 128:(kv + 1) * 128],
                            rhs=xts(t, half * 512, (half + 1) * 512),
                            start=(t == 0), stop=(t == KC - 1))
                for sb in range(4):
                    nc.tensor.matmul(
                        vps_a[sb][:],
                        lhsT=xts(t, sb * 128, (sb + 1) * 128),
                        rhs=wv_c(t),
                        start=(t == 0), stop=(t == KC - 1))

            kr_t = []
            for kv in range(KPC):
                kr = p_rt.tile([128, S], bf, tag="kr", name=f"kr{kv}")
                for half in range(2):
                    nc.scalar.copy(kr[:, half * 512:(half + 1) * 512],
                                   kps[kv * 2 + half][:])
                kr_t.append(kr)

            v_t = [p_v.tile([128, 2 * (HD + 1)], bf, tag="v",
                            name=f"v{sb}") for sb in range(SB)]

            def v_fin(sb, vp):
                tv = v_t[sb]
                nc.vector.tensor_copy(tv[:, 0:HD], vp[:, 0:HD])
                nc.vector.tensor_copy(tv[:, HD + 1:2 * HD + 1],
                                      vp[:, HD:2 * HD])
                nc.vector.memset(tv[:, HD:HD + 1], 1.0)
                nc.vector.memset(tv[:, 2 * HD + 1:2 * HD + 2], 1.0)

            for kv in range(KPC):
                rope_var(k1_t[kv], kr_t[kv])
                rope_negw(k2_t[kv], kr_t[kv])
            for sb in range(4):
                v_fin(sb, vps_a[sb])

            # ---- wave B: V sb4..7 + Q projections (x resident now) ----
            vps_b = [pst(256, f"vpsb{i}") for i in range(4)]
            for sb in range(4, SB):
                for t in range(KC):
                    nc.tensor.matmul(
                        vps_b[sb - 4][:],
                        lhsT=xts(t, sb * 128, (sb + 1) * 128),
                        rhs=wv_c(t),
                        start=(t == 0), stop=(t == KC - 1))

            q1_t, q2_t = [], []
            for h in range(HPC):
                d1 = p_q.tile([128, S], bf, tag="q", name=f"q1_{h}")
                d2 = p_q.tile([128, S], bf, tag="q", name=f"q2_{h}")
                pss = [pst(512, f"qps{h}{half}") for half in range(2)]
                for t in range(KC):
                    for half in range(2):
                        nc.tensor.matmul(
                            pss[half][:],
                            lhsT=wq_c(t)[:, h * 128:(h + 1) * 128],
                            rhs=xts(t, half * 512, (half + 1) * 512),
                            start=(t == 0), stop=(t == KC - 1))
                for half in range(2):
                    nc.scalar.copy(d2[:, half * 512:(half + 1) * 512],
                                   pss[half][:])
                rope_var(d1, d2)
                q1_t.append(d1)
                q2_t.append(d2)
                if h == 1:
                    for sb in range(4, SB):
                        v_fin(sb, vps_b[sb - 4])

            # ---- attention: per head, key-stationary wide scores ----
            ao_t = [p_ao.tile([128, S], bf, tag="ao", name=f"ao{h}")
                    for h in range(HPC)]

            def emit_scores(h):
                """P_j [128, (8-j)*128] = masked exp scores, q-blocks
                j..7 for key block j."""
                kv = h // 2
                Pt = []
                for j in range(SB):
                    nb = min(3, SB - j)          # band q-blocks j..j+2
                    wP = (SB - j) * 128
                    P = p_e.tile([128, wP], bf, tag="e", name=f"P{h}{j}")
                    psb = pst(nb * 128, f"sb{h}{j}")
                    nc.tensor.matmul(
                        psb[:], lhsT=k1_t[kv][:, j * 128:(j + 1) * 128],
                        rhs=q1_t[h][:, j * 128:(j + nb) * 128],
                        start=True, stop=True)
                    nc.scalar.activation(P[:, 0:nb * 128], psb[:],
                                         AF.Exp, scale=SCALE)
                    if j <= SB - 3:
                        # band part of overlap q-block j+2 (q < k)
                        ov = p_pt.tile([128, 128], bf, tag="pt")
                        nc.gpsimd.tensor_mul(ov[:], P[:, 256:384], m2_t)
                        # far scores q-blocks j+2..7 -> exp in place
                        nf = SB - j - 2
                        o = 0
                        while o < nf * 128:
                            wseg = min(512, nf * 128 - o)
                            psf = pst(wseg, f"sf{h}{j}{o}")
                            nc.tensor.matmul(
                                psf[:],
                                lhsT=k2_t[kv][:, j * 128:(j + 1) * 128],
                                rhs=q2_t[h][:, (j + 2) * 128 + o:
                                            (j + 2) * 128 + o + wseg],
                                start=True, stop=True)
                            nc.scalar.activation(
                                P[:, 256 + o:256 + o + wseg], psf[:],
                                AF.Exp, scale=SCALE)
                            o += wseg
                        # merge overlap: far part (k<=q) + band part
                        nc.gpsimd.tensor_mul(P[:, 256:384],
                                             P[:, 256:384], m0_t)
                        nc.gpsimd.tensor_add(P[:, 256:384],
                                             P[:, 256:384], ov[:])
                    # diagonal block: causal mask k<=q
                    nc.gpsimd.tensor_mul(P[:, 0:128], P[:, 0:128], m0_t)
                    Pt.append(P)
                return Pt

            def emit_attnv(i, h, Pt):
                kv = h // 2
                pso = pst(HD + 1, f"av{h}{i}")
                for j in range(i + 1):
                    nc.tensor.matmul(
                        pso[:],
                        lhsT=Pt[j][:, (i - j) * 128:(i - j + 1) * 128],
                        rhs=v_t[j][:, kv * (HD + 1):(kv + 1) * (HD + 1)],
                        start=(j == 0), stop=(j == i))
                rc = p_rc.tile([128, 1], f32, tag="rc")
                nc.vector.reciprocal(rc[:], pso[:, HD:HD + 1])
                an = p_pt.tile([128, 128], bf, tag="an")
                nc.vector.tensor_scalar_mul(an[:], pso[:, 0:HD], rc[:])
                # A^T via DMA transpose on the (idle) DMA rings
                nc.sync.dma_start_transpose(
                    ao_t[h][:, i * 128:(i + 1) * 128], an[:])

            def flush(row):
                for cg in range(4):
                    po = pst(512, f"out{row}{cg}")
                    for hc in range(HPC):
                        nc.tensor.matmul(
                            po[:],
                            lhsT=ao_t[hc][:, row * 128:(row + 1) * 128],
                            rhs=wo_s(hc, cg * 512, (cg + 1) * 512),
                            start=(hc == 0), stop=(hc == HPC - 1))
                    st = p_st.tile([128, 512], bf, tag="st")
                    if cg % 2 == 0:
                        nc.vector.tensor_copy(st[:], po[:])
                    else:
                        nc.scalar.copy(st[:], po[:])
                    nc.sync.dma_start(
                        out[row * 128:(row + 1) * 128,
                            cg * 512:(cg + 1) * 512], st[:])

            # scores of head h+1 overlap attn-V of head h; out-proj of
            # row i follows the last head's attn-V of row i
            Pts = [None] * HPC
            for h in range(HPC):
                Pts[h] = emit_scores(h)
                if h > 0:
                    for i in range(SB):
                        emit_attnv(i, h - 1, Pts[h - 1])
            for i in range(SB):
                emit_attnv(i, HPC - 1, Pts[HPC - 1])
                flush(i)

    nc.finalize()
    return nc


def _get_nc():
    if "nc" not in _NC_CACHE:
        _NC_CACHE["nc"] = _build_nc()
    return _NC_CACHE["nc"]


def _host_inputs(x, freqs_cos, freqs_sin, wq, wk, wv, wo):
    """Build the 8 per-core input maps (host-side shard + layout prep)."""
    x = np.asarray(x, np.float32)
    wq = np.asarray(wq, np.float32)
    wk = np.asarray(wk, np.float32)
    wv = np.asarray(wv, np.float32)
    wo = np.asarray(wo, np.float32)
    perm = np.concatenate([np.arange(0, HD, 2), np.arange(1, HD, 2)])

    cos_t = np.asarray(freqs_cos, np.float32).T        # (64, S)
    sin_t = np.asarray(freqs_sin, np.float32).T
    tab = np.concatenate([
        np.concatenate([cos_t, -sin_t], axis=1),       # rows 0:64
        np.concatenate([cos_t, sin_t], axis=1),        # rows 64:128
    ], axis=0)                                         # (128, 2S)
    ki = np.arange(128)[:, None]
    qi = np.arange(128)[None, :]
    m0 = (ki <= qi).astype(np.float32)                 # causal / far-select
    m2 = (qi < ki).astype(np.float32)                  # in-band select

    wq3 = wq.reshape(D, NH, HD)
    wk3 = wk.reshape(D, NKV, HD)
    wv3 = wv.reshape(D, NKV, HD)
    wo3 = wo.reshape(NH, HD, D)

    cwh = np.stack([cos_t[:, W], sin_t[:, W], -sin_t[:, W]],
                   axis=1)                              # (64, 3)
    cw = np.concatenate([cwh, cwh], axis=0).astype(np.float32)

    in_maps = []
    for c in range(8):
        b, g = divmod(c, 4)
        wqc = wq3[:, 4 * g:4 * g + 4][:, :, perm].reshape(D, HPC * HD)
        wkc = wk3[:, 2 * g:2 * g + 2][:, :, perm].reshape(D, KPC * HD)
        wvc = wv3[:, 2 * g:2 * g + 2].reshape(D, KPC * HD)
        woc = wo3[4 * g:4 * g + 4].reshape(HPC * HD, D)
        xt = x[b].T                                     # (D, S)

        blob = np.empty((128, NBLOB), np.float32)
        blob[:, O_M0:O_M0 + 128] = m0
        blob[:, O_M2:O_M2 + 128] = m2
        blob[:, O_TAB:O_TAB + 2 * S] = tab
        for gi in range(8):
            o = O_GRP + gi * GRPW
            for e in range(2):
                t = 2 * gi + e
                rs = slice(t * 128, (t + 1) * 128)
                blob[:, o + e * 256:o + (e + 1) * 256] = wkc[rs]
                blob[:, o + 512 + e * 256:
                     o + 512 + (e + 1) * 256] = wvc[rs]
                blob[:, o + 1024 + e * 512:
                     o + 1024 + (e + 1) * 512] = wqc[rs]
                blob[:, o + 2048 + e * 1024:
                     o + 2048 + (e + 1) * 1024] = xt[rs]
        blob[:, O_WO:NBLOB] = woc.reshape(HPC, HD, D).transpose(
            1, 0, 2).reshape(128, HPC * D)
        in_maps.append({
            "blob": np.ascontiguousarray(blob).astype(BF16),
            "cw": cw,
        })
    return in_maps


def _run(nc, in_maps, **kw):
    from concourse.bass_utils import run_bass_kernel_spmd
    return run_bass_kernel_spmd(nc, in_maps, core_ids=list(range(8)), **kw)


def kernel(x, freqs_cos, freqs_sin, wq, wk, wv, wo):
    nc = _get_nc()
    in_maps = _host_inputs(x, freqs_cos, freqs_sin, wq, wk, wv, wo)
    res = _run(nc, in_maps)
    parts = [np.asarray(res.results[c]["out"], np.float32) for c in range(8)]
    out = np.stack([sum(parts[0:4]), sum(parts[4:8])])
    return out.astype(np.float32)
